# revision 1
# baseline (speedup 1.0000x reference)
"""GatedAttentionBlock kernel sharded across 8 NeuronCores.

Sharding: 8 shards = (batch b in {0,1}) x (query-sequence chunk c in {0..3}).
Each core holds the full x (needed for K/V over all positions) and computes
its 512-row query chunk end-to-end: rmsnorm -> qkv -> Householder-RoPE ->
causal attention -> out proj -> sigmoid gate -> residual -> rmsnorm -> SwiGLU
-> residual.  Rows are independent outside attention, and attention only needs
full K/V (computed locally from the replicated x), so no collectives are
required; the host concatenates the 8 output shards.

Weights and mask are device_put_replicated once and cached, so repeat calls
only transfer x.
"""
import numpy as np
import jax
import jax.numpy as jnp

B, S, D, H = 2, 2048, 1024, 16
HD = D // H            # 64
NC = 8                 # cores
CHUNKS = 4             # sequence chunks per batch element
SC = S // CHUNKS       # 512 rows per shard


def _householder(vs):
    def step(Q, v):
        v = v[:, None]
        Q = Q - (2.0 / (jnp.sum(v * v) + 1e-8)) * (v @ (v.T @ Q))
        return Q, None
    Q, _ = jax.lax.scan(step, jnp.eye(vs.shape[-1], dtype=vs.dtype), vs)
    return Q


def _rmsnorm(x):
    return x * jax.lax.rsqrt(jnp.mean(x * x, axis=-1, keepdims=True)
                             + jnp.finfo(x.dtype).eps)


def _shard_fn(b_idx, start, x, mask, qkv_w, out_w, gate_w, gate_b,
              w12, w3, hh_vs, inv_freq, rope_pos):
    # x [B,S,D] full input; this shard handles batch b_idx, query rows
    # [start, start+SC).
    x_b = jax.lax.dynamic_index_in_dim(x, b_idx, axis=0, keepdims=False)
    mask_rows = jax.lax.dynamic_slice_in_dim(mask, start, SC, axis=0)

    xn = _rmsnorm(x_b)
    qkv = xn @ qkv_w.T                                     # [S,3D]
    q, k, v = jnp.split(qkv, 3, axis=-1)
    q = q.reshape(S, H, HD).transpose(1, 0, 2)             # [H,S,HD]
    k = k.reshape(S, H, HD).transpose(1, 0, 2)
    v = v.reshape(S, H, HD).transpose(1, 0, 2)

    Q = _householder(hh_vs)
    q = q @ Q.T
    k = k @ Q.T

    full = jnp.einsum('sd,f->sdf', rope_pos, inv_freq).reshape(S, -1)
    full = full[:, :HD // 2]
    emb = jnp.concatenate([full, full], axis=-1)           # [S,HD]
    cos, sin = jnp.cos(emb), jnp.sin(emb)

    def rot(t, c, s):
        t1, t2 = jnp.split(t, 2, axis=-1)
        return t * c + jnp.concatenate([-t2, t1], axis=-1) * s

    q_c = jax.lax.dynamic_slice_in_dim(q, start, SC, axis=1)   # [H,SC,HD]
    cos_c = jax.lax.dynamic_slice_in_dim(cos, start, SC, axis=0)
    sin_c = jax.lax.dynamic_slice_in_dim(sin, start, SC, axis=0)
    qr = rot(q_c, cos_c, sin_c) @ Q
    kr = rot(k, cos, sin) @ Q

    scores = jnp.einsum('hsd,htd->hst', qr, kr) / jnp.sqrt(
        jnp.asarray(HD, x.dtype))
    scores = jnp.where(mask_rows[None], scores, -jnp.inf)
    attn = jax.nn.softmax(scores, axis=-1)
    o = jnp.einsum('hst,htd->hsd', attn, v)                # [H,SC,HD]
    o = o.transpose(1, 0, 2).reshape(SC, D)
    o = o @ out_w.T

    resid = jax.lax.dynamic_slice_in_dim(x_b, start, SC, axis=0)
    gate = jax.nn.sigmoid(o @ gate_w.T + gate_b)
    x2_ = resid + o * gate

    xn2 = _rmsnorm(x2_)
    x12 = xn2 @ w12.T
    a, b = jnp.split(x12, 2, axis=-1)
    ffn = (jax.nn.silu(a) * b) @ w3.T
    return x2_ + ffn                                       # [SC,D]


_CACHE = {}


def kernel(x, mask, qkv_w, out_w, gate_w, gate_b, w12, w3,
           hh_vs, inv_freq, rope_pos):
    x = np.asarray(x, np.float32)
    mask = np.asarray(mask, bool)
    devs = jax.devices()
    if len(devs) >= NC:
        devs = devs[:NC]
        wkey = (id(mask), id(qkv_w), id(out_w), id(gate_w), id(gate_b),
                id(w12), id(w3), id(hh_vs), id(inv_freq), id(rope_pos))
        if _CACHE.get("wkey") != wkey:
            _CACHE["wkey"] = wkey
            _CACHE["consts"] = tuple(
                jax.device_put_replicated(np.asarray(a), devs)
                for a in (mask, qkv_w, out_w, gate_w, gate_b, w12, w3,
                          hh_vs, inv_freq, rope_pos))
            _CACHE["b_idx"] = jax.device_put_sharded(
                [np.int32(i // CHUNKS) for i in range(NC)], devs)
            _CACHE["start"] = jax.device_put_sharded(
                [np.int32((i % CHUNKS) * SC) for i in range(NC)], devs)
            _CACHE["fn"] = jax.pmap(_shard_fn, devices=devs)
        xr = jax.device_put_replicated(x, devs)
        out = _CACHE["fn"](_CACHE["b_idx"], _CACHE["start"], xr,
                           *_CACHE["consts"])
        out = np.asarray(out)                              # [8,SC,D]
        return out.reshape(B, CHUNKS, SC, D).reshape(B, S, D).astype(np.float32)

    # Single-device fallback.
    if "jit" not in _CACHE:
        def _full(x, mask, *ws):
            outs = []
            for b in range(B):
                rows = [
                    _shard_fn(jnp.int32(b), jnp.int32(c * SC), x, mask, *ws)
                    for c in range(CHUNKS)]
                outs.append(jnp.concatenate(rows, axis=0))
            return jnp.stack(outs)
        _CACHE["jit"] = jax.jit(_full)
    out = _CACHE["jit"](jnp.asarray(x), jnp.asarray(mask), jnp.asarray(qkv_w),
                        jnp.asarray(out_w), jnp.asarray(gate_w),
                        jnp.asarray(gate_b), jnp.asarray(w12),
                        jnp.asarray(w3), jnp.asarray(hh_vs),
                        jnp.asarray(inv_freq), jnp.asarray(rope_pos))
    return np.asarray(out, np.float32)

